# revision 24
# baseline (speedup 1.0000x reference)
"""Trainium2 Bass kernel for nn_AttentionModified (MQA-over-variants attention).

Strategy: data-parallel over B across 8 NeuronCores (no collectives — each
batch's output depends only on that batch's inputs).

Per-core pipeline (bf16 compute, f32 PSUM accumulation):
  - activations pre-transposed on host (x^T, vp^T) -> natural contiguous DMA
    loads (the on-device transpose-DMA wall of the earlier design is gone)
  - fused KV projection: one matmul chain per variant produces [v(64); k(64)]
    on 128 psum partitions (the M dim is free on the PE); k is then duplicated
    to the [k; k] layout QK needs via SBUF->SBUF DMAs issued from the scalar
    queue; v rows are consumed in place as a view
  - QK logits: one broadcast-AP vector multiply per k-chunk (q repeated over
    all 8 variants), then block-ones matmuls reduce 64-wide head groups -> s^T
  - softmax token-major: exp on the [12, 512] logit tiles, then cheap 12-col
    PE transposes -> [128 tok, (v,h)] psum; Z-reduce, reciprocal and the 1/Z
    scaling all happen on [128, 96] tiles (two orders of magnitude less work
    than head-major); per-variant head-replication to [128, 768] is split
    between the scalar and gpsimd engines
  - AV: paired broadcast-AP vector multiplies + a short add tree
  - output projection; bias fused into PSUM eviction; output written
    transposed, host transposes back
Emission order software-pipelines the two 512-token halves so PE projection
work for half h+1 fills the gaps in the DVE-bound attention phase of half h.
"""
import sys

sys.path.insert(0, "/opt/trn_rl_repo")

import numpy as np
import ml_dtypes

import concourse.bass as bass
import concourse.mybir as mybir
import concourse.tile as tile
from concourse.bass_utils import run_bass_kernel_spmd

BF16 = mybir.dt.bfloat16
F32 = mybir.dt.float32
BF = ml_dtypes.bfloat16

V, B, N, C, H = 8, 8, 1024, 768, 12
HD = C // H  # 64
NK = C // 128  # 6 contraction chunks
HALF = 512
SCALE = HD ** -0.5


def _split_multi_waits(nc):
    """This container's walrus accepts only one sync-wait per instruction;
    hoist extra waits onto same-engine NoOps inserted just before."""
    for f in nc.m.functions:
        for bb in f.blocks:
            new = []
            for inst in bb.instructions:
                si = inst.sync_info
                waits = list(si.on_wait) if (si and si.on_wait) else []
                if len(waits) > 1:
                    for i, w in enumerate(waits[:-1]):
                        nop = mybir.InstNoOp(name=f"{inst.name}-wsplit{i}")
                        nop.engine = inst.engine
                        nop.sync_info = mybir.SyncInfo(on_wait=[w], on_update=[])
                        new.append(nop)
                    si.on_wait = [waits[-1]]
                new.append(inst)
            bb.instructions[:] = new
    return nc


def _bc(a, dims):
    """Rebuild AP `a` with an explicit dim list (partition dim first)."""
    return bass.AP(tensor=a.tensor, offset=a.offset, ap=dims)


def build_kernel():
    nc = bass.Bass("TRN2", target_bir_lowering=False, debug=False, num_devices=8)

    xw = nc.dram_tensor("xw", [C, N], BF16, kind="ExternalInput").ap()
    vp = nc.dram_tensor("vp", [V, C, N], BF16, kind="ExternalInput").ap()
    wq = nc.dram_tensor("wq", [C, C], BF16, kind="ExternalInput").ap()
    wkv = nc.dram_tensor("wkv", [C, 128], BF16, kind="ExternalInput").ap()
    wp = nc.dram_tensor("wp", [C, C], BF16, kind="ExternalInput").ap()
    bp = nc.dram_tensor("bp", [C, 1], F32, kind="ExternalInput").ap()
    ones = nc.dram_tensor("ones", [C, H], BF16, kind="ExternalInput").ap()
    ident = nc.dram_tensor("ident", [128, 128], BF16, kind="ExternalInput").ap()
    id12 = nc.dram_tensor("id12", [12, 12], BF16, kind="ExternalInput").ap()
    outt = nc.dram_tensor("outt", [C, N], F32, kind="ExternalOutput").ap()

    EXP = mybir.ActivationFunctionType.Exp
    IDENT = mybir.ActivationFunctionType.Identity

    with tile.TileContext(nc) as tc:
        with (
            tc.tile_pool(name="singles", bufs=1) as singles,
            tc.tile_pool(name="vtp", bufs=3) as vtp_pool,
            tc.tile_pool(name="kvp", bufs=2) as kv_pool,
            tc.tile_pool(name="ktp", bufs=2) as kt_pool,
            tc.tile_pool(name="acts", bufs=2) as acts,
            tc.tile_pool(name="acts1", bufs=2) as acts1,
            tc.tile_pool(name="tmp", bufs=2) as tmp_pool,
            tc.tile_pool(name="sm", bufs=2) as sm_pool,
            tc.tile_pool(name="etx", bufs=3) as etx_pool,
            tc.tile_pool(name="av", bufs=1) as av_pool,
            tc.tile_pool(name="of", bufs=2) as of_pool,
            tc.tile_pool(name="outp", bufs=2) as out_pool,
            tc.tile_pool(name="psmm", bufs=3, space="PSUM") as psum_mm,
            tc.tile_pool(name="psss", bufs=2, space="PSUM") as psum_s,
            tc.tile_pool(name="pstr", bufs=3, space="PSUM") as psum_tr,
        ):
            # ---- constants (emission order = sync-ring order: small KV weight
            # first, then vp chunks so the fused KV proj can start early)
            wkv_sb = singles.tile([128, NK, 128], BF16)
            nc.sync.dma_start(out=wkv_sb[:], in_=wkv.rearrange("(j p) o -> p j o", p=128))
            ones_sb = singles.tile([128, NK, H], BF16)
            nc.sync.dma_start(out=ones_sb[:], in_=ones.rearrange("(j p) o -> p j o", p=128))
            id_sb = singles.tile([128, 128], BF16)
            nc.sync.dma_start(out=id_sb[:], in_=ident)
            id12_sb = singles.tile([12, 12], BF16)
            nc.sync.dma_start(out=id12_sb[:], in_=id12)
            wq_sb = singles.tile([128, NK, C], BF16)
            wp_sb = singles.tile([128, NK, C], BF16)
            bp_sb = singles.tile([128, NK], F32)
            xt_sb = singles.tile([128, NK, N], BF16)

            xr = xw.rearrange("(j p) n -> p j n", p=128)
            vpr = vp.rearrange("v (j p) n -> p j v n", p=128)

            def emit_late_consts():
                nc.sync.dma_start(out=xt_sb[:, :, HALF:N], in_=xr[:, :, HALF:N])
                nc.sync.dma_start(out=wp_sb[:], in_=wp.rearrange("(j p) o -> p j o", p=128))
                nc.sync.dma_start(out=bp_sb[:], in_=bp.rearrange("(j p) 1 -> p j", p=128))

            def emit_vp_loads(h2, g):
                # one tile per 4-variant group; short lifetime (KV proj only)
                T0 = h2 * HALF
                vpt = vtp_pool.tile([128, NK, 4, HALF], BF16, tag="vpg", name="vpg")
                for i in range(4):
                    nc.sync.dma_start(
                        out=vpt[:, :, i, :], in_=vpr[:, :, 4 * g + i, T0 : T0 + HALF]
                    )
                return vpt

            def emit_q_chunks(qt, h2, ms):
                T0 = h2 * HALF
                for m in ms:
                    psq = psum_mm.tile([128, HALF], F32, tag="mm", name="psq")
                    for k in range(NK):
                        nc.tensor.matmul(
                            psq[:],
                            lhsT=wq_sb[:, k, m * 128 : (m + 1) * 128],
                            rhs=xt_sb[:, k, T0 : T0 + HALF],
                            start=(k == 0),
                            stop=(k == NK - 1),
                        )
                    nc.scalar.copy(qt[:, m, :], psq[:])

            def emit_q(h2):
                qt = acts.tile([128, NK, HALF], BF16, tag="qt", name="qt")
                emit_q_chunks(qt, h2, range(NK))
                return qt

            def emit_kv_group(kvt, vpt, kv0, vp0, gn=2):
                # fused K+V: psum rows 0:64 = v, 64:128 = k (wkv pre-concat on
                # host); two parallel chains keep the PE streaming
                psks = [
                    psum_mm.tile([128, HALF], F32, tag="mm", name=f"pskv{i}")
                    for i in range(gn)
                ]
                for k in range(NK):
                    for i in range(gn):
                        nc.tensor.matmul(
                            psks[i][:],
                            lhsT=wkv_sb[:, k, :],
                            rhs=vpt[:, k, vp0 + i, :],
                            start=(k == 0),
                            stop=(k == NK - 1),
                        )
                for i in range(gn):
                    nc.scalar.copy(kvt[:, kv0 + i, :], psks[i][:])

            def emit_kt_dup(kvt, kt, vs0, vn):
                # duplicate the k rows (64:128 of kv) into both halves of kt
                nc.scalar.dma_start(
                    out=kt[0:64, vs0 : vs0 + vn, :], in_=kvt[64:128, vs0 : vs0 + vn, :]
                )
                nc.scalar.dma_start(
                    out=kt[64:128, vs0 : vs0 + vn, :], in_=kvt[64:128, vs0 : vs0 + vn, :]
                )

            def emit_tile(tt, qt, kt, kvt, ot, sc_etx=(0, 2, 4, 6)):
                t0 = tt * 128
                # v natural: transpose v rows (kv[0:64]) per variant
                psvn = psum_tr.tile([128, C], BF16, tag="tr", name="psvn")
                for v in range(V):
                    nc.tensor.transpose(
                        psvn[:, v * HD : (v + 1) * HD],
                        kvt[0:64, v, t0 : t0 + 128],
                        id_sb[0:64, 0:64],
                    )
                vnat = sm_pool.tile([128, V, HD], BF16, name="vnat")
                nc.scalar.copy(vnat[:], psvn[:, 0 : V * HD])

                # QK -> s^T   (one 8-variant broadcast mul per k-chunk)
                psst = psum_s.tile([44, 512], F32, tag="ss", name="pss")
                pss1 = psst[0:12, :]
                pss2 = psst[32:44, :]
                for j in range(NK):
                    qa = qt[:, j, t0 : t0 + 128]
                    q_b = _bc(qa, [qa.ap[0], [0, V], qa.ap[-1]])
                    tmpa = tmp_pool.tile([128, V, 128], BF16, tag="tmpa", name="tmpa")
                    nc.vector.tensor_mul(tmpa[:], q_b, kt[:, :, t0 : t0 + 128])
                    nc.tensor.matmul(
                        pss1,
                        lhsT=ones_sb[:, j, :],
                        rhs=tmpa[:, 0:4, :],
                        start=(j == 0),
                        stop=(j == NK - 1),
                        tile_position=(0, 0),
                    )
                    nc.tensor.matmul(
                        pss2,
                        lhsT=ones_sb[:, j, :],
                        rhs=tmpa[:, 4:8, :],
                        start=(j == 0),
                        stop=(j == NK - 1),
                        tile_position=(0, 32),
                    )

                # softmax, token-major: exp -> e [12, V*128]; transpose each
                # variant's [12, 128] slab to [128, 12] -> eta [128 tok, (v,h)]
                e = sm_pool.tile([12, V * 128], BF16, name="e")
                nc.scalar.activation(e[:, 0:512], pss1, EXP, scale=SCALE)
                nc.scalar.activation(e[:, 512:1024], pss2, EXP, scale=SCALE)
                eta = psum_tr.tile([128, C], BF16, tag="tr", name="eta")
                for v in range(V):
                    nc.tensor.transpose(
                        eta[:, v * H : (v + 1) * H],
                        e[:, v * 128 : (v + 1) * 128],
                        id12_sb[:],
                    )
                z = sm_pool.tile([128, H], F32, name="z")
                ea = eta[:, 0 : V * H]
                e_sw = _bc(ea, [ea.ap[0], [1, H], [H, V]])
                nc.vector.tensor_reduce(
                    z[:], e_sw, axis=mybir.AxisListType.X, op=mybir.AluOpType.add
                )
                rz = sm_pool.tile([128, H], F32, name="rz")
                nc.vector.reciprocal(rz[:], z[:])
                # fold 1/Z into the (tiny) token-major attention tile
                et2 = sm_pool.tile([128, V * H], BF16, name="et2")
                rza = rz[:]
                rz_b = _bc(rza, [rza.ap[0], [0, V], rza.ap[-1]])
                nc.vector.tensor_mul(et2[:], eta[:, 0 : V * H], rz_b)

                # head-replicate per variant-pair: [128, 12] -> [128, 768];
                # even variant on scalar, odd variant on gpsimd
                def rep_ap(v):
                    a = et2[:, v * H : (v + 1) * H]
                    return _bc(a, [a.ap[0], a.ap[-1], [0, HD]])

                ovA = av_pool.tile([128, 4, C], BF16, tag="ovA", name="ovA")
                ovB = av_pool.tile([128, 4, C], BF16, tag="ovB", name="ovB")
                ovs = (ovA, ovB)
                for p in range(4):
                    exp_p = etx_pool.tile([128, 2, C], BF16, tag="etx", name=f"etx{p}")
                    v0 = 2 * p
                    if v0 in sc_etx:
                        nc.scalar.copy(exp_p[:, 0, :], rep_ap(v0))
                    else:
                        nc.gpsimd.tensor_copy(exp_p[:, 0, :], rep_ap(v0))
                    if (v0 + 1) in sc_etx:
                        nc.scalar.copy(exp_p[:, 1, :], rep_ap(v0 + 1))
                    else:
                        nc.gpsimd.tensor_copy(exp_p[:, 1, :], rep_ap(v0 + 1))
                    # paired AV multiply (both variants in one DVE op)
                    va = vnat[:, v0 : v0 + 2, :]
                    vn_b = _bc(va, [va.ap[0], va.ap[1], [0, H], va.ap[-1]])
                    ea_p = exp_p[:]
                    ex_b = _bc(ea_p, [ea_p.ap[0], ea_p.ap[1], [HD, H], [1, HD]])
                    ov = ovs[p // 2]
                    oslc = ov[:, (p % 2) * 2 : (p % 2) * 2 + 2, :]
                    ov_w = _bc(oslc, [oslc.ap[0], oslc.ap[1], [HD, H], [1, HD]])
                    nc.vector.tensor_mul(ov_w, vn_b, ex_b)
                # add tree: one paired level-1 add per ov tile, then combine
                for ov in ovs:
                    a0 = ov[:, 0:2:1, :]
                    in0 = _bc(a0, [a0.ap[0], [2 * C, 2], [1, C]])
                    a1 = ov[:, 1:3:1, :]
                    in1 = _bc(a1, [a1.ap[0], [2 * C, 2], [1, C]])
                    nc.vector.tensor_add(in0, in0, in1)
                    nc.vector.tensor_add(ov[:, 0, :], ov[:, 0, :], ov[:, 2, :])
                of = of_pool.tile([128, C], BF16, tag="of", name="of")
                nc.vector.tensor_add(of[:], ovA[:, 0, :], ovB[:, 0, :])

                # o^T
                psot = psum_tr.tile([128, C], BF16, tag="tr", name="psot")
                for j in range(NK):
                    nc.tensor.transpose(
                        psot[:, j * 128 : (j + 1) * 128],
                        of[:, j * 128 : (j + 1) * 128],
                        id_sb[:],
                    )
                oa = psot[:]
                nc.scalar.copy(
                    ot[:, :, t0 : t0 + 128], _bc(oa, [oa.ap[0], [128, NK], [1, 128]])
                )

            def emit_projout(h2, ot, ms=None):
                T0 = h2 * HALF
                for m in (range(NK) if ms is None else ms):
                    pso2 = psum_mm.tile([128, HALF], F32, tag="mm", name="pso2")
                    for k in range(NK):
                        nc.tensor.matmul(
                            pso2[:],
                            lhsT=wp_sb[:, k, m * 128 : (m + 1) * 128],
                            rhs=ot[:, k, :],
                            start=(k == 0),
                            stop=(k == NK - 1),
                        )
                    o2 = out_pool.tile([128, HALF], BF16, name="o2")
                    nc.scalar.activation(
                        o2[:], pso2[:], IDENT, bias=bp_sb[:, m : m + 1], scale=1.0
                    )
                    nc.gpsimd.dma_start(
                        out=outt[m * 128 : (m + 1) * 128, T0 : T0 + HALF], in_=o2[:]
                    )

            def emit_kv(h2, vpt, kvt=None, kt=None, phase=0):
                # vpt holds 4 variants (local idx 0..3) for group phase//2
                if kvt is None:
                    kvt = kv_pool.tile([128, V, HALF], BF16, tag="kv", name="kv")
                    kt = kt_pool.tile([128, V, HALF], BF16, tag="kt", name="kt")
                g0 = 0 if phase == 0 else 4
                emit_kv_group(kvt, vpt, g0, 0)
                emit_kv_group(kvt, vpt, g0 + 2, 2)
                emit_kt_dup(kvt, kt, g0, 4)
                return kvt, kt

            # ---- software-pipelined schedule over the two halves
            vpt00 = emit_vp_loads(0, 0)
            nc.sync.dma_start(out=xt_sb[:, :, 0:HALF], in_=xr[:, :, 0:HALF])
            nc.sync.dma_start(out=wq_sb[:], in_=wq.rearrange("(j p) o -> p j o", p=128))
            vpt01 = emit_vp_loads(0, 1)
            emit_late_consts()
            qt0 = acts.tile([128, NK, HALF], BF16, tag="qt", name="qt0")
            kv0, kt0 = emit_kv(0, vpt00, phase=0)
            emit_q_chunks(qt0, 0, [0])
            emit_kv(0, vpt01, kv0, kt0, phase=2)
            emit_q_chunks(qt0, 0, [1, 2, 3, 4, 5])
            vpt10 = emit_vp_loads(1, 0)
            vpt11 = emit_vp_loads(1, 1)
            ot0 = acts1.tile([128, NK, HALF], BF16, tag="ot", name="ot0")
            qt1 = kv1 = kt1 = None
            for tt in range(4):
                emit_tile(tt, qt0, kt0, kv0, ot0)
                if tt == 0:
                    qt1 = emit_q(1)
                elif tt == 1:
                    kv1, kt1 = emit_kv(1, vpt10, phase=0)
                elif tt == 2:
                    emit_kv(1, vpt11, kv1, kt1, phase=2)
            ot1 = acts1.tile([128, NK, HALF], BF16, tag="ot", name="ot1")
            emit_tile(0, qt1, kt1, kv1, ot1)
            emit_projout(0, ot0, [0, 1])
            emit_tile(1, qt1, kt1, kv1, ot1)
            emit_projout(0, ot0, [2, 3])
            emit_tile(2, qt1, kt1, kv1, ot1)
            emit_projout(0, ot0, [4, 5])
            emit_tile(3, qt1, kt1, kv1, ot1)
            emit_projout(1, ot1)

    _split_multi_waits(nc)
    return nc


_NC = None


def _get_nc():
    global _NC
    if _NC is None:
        _NC = build_kernel()
    return _NC


def _host_inputs(x, variants_patches, Wq, Wkv, Wproj, bproj):
    wq_t = np.ascontiguousarray(np.asarray(Wq, dtype=np.float32).T).astype(BF)
    wkv_np = np.asarray(Wkv, dtype=np.float32)
    # fused KV: psum rows 0:64 = v outputs, 64:128 = k outputs
    wkvcat = np.ascontiguousarray(
        np.concatenate([wkv_np[HD:], wkv_np[:HD]], axis=0).T
    ).astype(BF)
    wp_t = np.ascontiguousarray(np.asarray(Wproj, dtype=np.float32).T).astype(BF)
    bp = np.asarray(bproj, dtype=np.float32).reshape(C, 1)
    ones = np.zeros((C, H), dtype=np.float32)
    for c in range(C):
        ones[c, c // HD] = 1.0
    ones = ones.astype(BF)
    ident = np.eye(128, dtype=np.float32).astype(BF)
    id12_np = np.eye(12, dtype=np.float32).astype(BF)

    x = np.asarray(x, dtype=np.float32)
    vpn = np.asarray(variants_patches, dtype=np.float32)
    # pre-transpose activations on host: x -> [C, N], vp -> [V, C, N]
    xt = np.ascontiguousarray(x.transpose(0, 2, 1)).astype(BF)  # (B, C, N)
    vpt = np.ascontiguousarray(vpn.transpose(1, 0, 3, 2)).astype(BF)  # (B, V, C, N)
    in_maps = []
    for b in range(B):
        in_maps.append(
            {
                "xw": xt[b],
                "vp": vpt[b],
                "wq": wq_t,
                "wkv": wkvcat,
                "wp": wp_t,
                "bp": bp,
                "ones": ones,
                "ident": ident,
                "id12": id12_np,
            }
        )
    return in_maps


def run(inputs, trace=False):
    nc = _get_nc()
    in_maps = _host_inputs(
        inputs["x"],
        inputs["variants_patches"],
        inputs["Wq"],
        inputs["Wkv"],
        inputs["Wproj"],
        inputs["bproj"],
    )
    res = run_bass_kernel_spmd(nc, in_maps, core_ids=list(range(8)), trace=trace)
    out = np.stack(
        [np.asarray(res.results[b]["outt"]).T for b in range(B)], axis=0
    ).astype(np.float32)
    return out, res


def kernel(**inputs) -> np.ndarray:
    out, _ = run(inputs, trace=False)
    return out


if __name__ == "__main__":
    rng = np.random.default_rng(0)
    ins = {
        "x": rng.standard_normal((B, N, C)).astype(np.float32),
        "variants_patches": rng.standard_normal((V, B, N, C)).astype(np.float32),
        "Wq": (rng.standard_normal((C, C)) * 0.02).astype(np.float32),
        "Wkv": (rng.standard_normal((2 * HD, C)) * 0.02).astype(np.float32),
        "Wproj": (rng.standard_normal((C, C)) * 0.02).astype(np.float32),
        "bproj": np.zeros((C,), dtype=np.float32),
        "num_layer": 0,
    }
    out = kernel(**ins)
    print("kernel ran, out shape", out.shape)


# revision 31
# speedup vs baseline: 1.0965x; 1.0965x over previous
"""Trainium2 Bass kernel for nn_AttentionModified (MQA-over-variants attention).

Strategy: data-parallel over B across 8 NeuronCores (no collectives — each
batch's output depends only on that batch's inputs).

Per-core pipeline (bf16 compute, f32 PSUM accumulation):
  - activations pre-transposed on host (x^T, vp^T) -> natural contiguous DMA
    loads (the on-device transpose-DMA wall of the earlier design is gone)
  - fused KV projection: one matmul chain per variant produces [v(64); k(64)]
    on 128 psum partitions (the M dim is free on the PE); k is then duplicated
    to the [k; k] layout QK needs via SBUF->SBUF DMAs issued from the scalar
    queue; v rows are consumed in place as a view
  - QK logits: one broadcast-AP vector multiply per k-chunk (q repeated over
    all 8 variants), then block-ones matmuls reduce 64-wide head groups -> s^T
  - softmax token-major: exp on the [12, 512] logit tiles, then cheap 12-col
    PE transposes -> [128 tok, (v,h)] psum; Z-reduce, reciprocal and the 1/Z
    scaling all happen on [128, 96] tiles (two orders of magnitude less work
    than head-major); per-variant head-replication to [128, 768] is split
    between the scalar and gpsimd engines
  - AV: paired broadcast-AP vector multiplies + a short add tree
  - output projection; bias fused into PSUM eviction; output written
    transposed, host transposes back
Emission order software-pipelines the two 512-token halves so PE projection
work for half h+1 fills the gaps in the DVE-bound attention phase of half h.
"""
import sys

sys.path.insert(0, "/opt/trn_rl_repo")

import numpy as np
import ml_dtypes

import concourse.bass as bass
import concourse.mybir as mybir
import concourse.tile as tile
from concourse.bass_utils import run_bass_kernel_spmd

BF16 = mybir.dt.bfloat16
F32 = mybir.dt.float32
BF = ml_dtypes.bfloat16

V, B, N, C, H = 8, 8, 1024, 768, 12
HD = C // H  # 64
NK = C // 128  # 6 contraction chunks
HALF = 512
SCALE = HD ** -0.5


def _split_multi_waits(nc):
    """This container's walrus accepts only one sync-wait per instruction;
    hoist extra waits onto same-engine NoOps inserted just before."""
    for f in nc.m.functions:
        for bb in f.blocks:
            new = []
            for inst in bb.instructions:
                si = inst.sync_info
                waits = list(si.on_wait) if (si and si.on_wait) else []
                if len(waits) > 1:
                    for i, w in enumerate(waits[:-1]):
                        nop = mybir.InstNoOp(name=f"{inst.name}-wsplit{i}")
                        nop.engine = inst.engine
                        nop.sync_info = mybir.SyncInfo(on_wait=[w], on_update=[])
                        new.append(nop)
                    si.on_wait = [waits[-1]]
                new.append(inst)
            bb.instructions[:] = new
    return nc


def _bc(a, dims):
    """Rebuild AP `a` with an explicit dim list (partition dim first)."""
    return bass.AP(tensor=a.tensor, offset=a.offset, ap=dims)


def build_kernel():
    nc = bass.Bass("TRN2", target_bir_lowering=False, debug=False, num_devices=8)

    xw = nc.dram_tensor("xw", [C, N], BF16, kind="ExternalInput").ap()
    vp = nc.dram_tensor("vp", [V, C, N], BF16, kind="ExternalInput").ap()
    wq = nc.dram_tensor("wq", [C, C], BF16, kind="ExternalInput").ap()
    wkv = nc.dram_tensor("wkv", [C, 128], BF16, kind="ExternalInput").ap()
    wp = nc.dram_tensor("wp", [C, C], BF16, kind="ExternalInput").ap()
    bp = nc.dram_tensor("bp", [C, 1], F32, kind="ExternalInput").ap()
    ones = nc.dram_tensor("ones", [C, H], BF16, kind="ExternalInput").ap()
    ident = nc.dram_tensor("ident", [128, 128], BF16, kind="ExternalInput").ap()
    id12 = nc.dram_tensor("id12", [12, 12], BF16, kind="ExternalInput").ap()
    repl = nc.dram_tensor("repl", [12, C], BF16, kind="ExternalInput").ap()
    outt = nc.dram_tensor("outt", [C, N], F32, kind="ExternalOutput").ap()

    EXP = mybir.ActivationFunctionType.Exp
    IDENT = mybir.ActivationFunctionType.Identity

    with tile.TileContext(nc) as tc:
        with (
            tc.tile_pool(name="singles", bufs=1) as singles,
            tc.tile_pool(name="vtp", bufs=3) as vtp_pool,
            tc.tile_pool(name="kvp", bufs=2) as kv_pool,
            tc.tile_pool(name="ktp", bufs=2) as kt_pool,
            tc.tile_pool(name="acts", bufs=2) as acts,
            tc.tile_pool(name="acts1", bufs=2) as acts1,
            tc.tile_pool(name="tmp", bufs=2) as tmp_pool,
            tc.tile_pool(name="sm", bufs=2) as sm_pool,
            tc.tile_pool(name="av", bufs=1) as av_pool,
            tc.tile_pool(name="of", bufs=2) as of_pool,
            tc.tile_pool(name="outp", bufs=2) as out_pool,
            tc.tile_pool(name="psmm", bufs=3, space="PSUM") as psum_mm,
            tc.tile_pool(name="psss", bufs=2, space="PSUM") as psum_s,
            tc.tile_pool(name="pstr", bufs=3, space="PSUM") as psum_tr,
        ):
            # ---- constants (emission order = sync-ring order: small KV weight
            # first, then vp chunks so the fused KV proj can start early)
            wkv_sb = singles.tile([128, NK, 128], BF16)
            nc.sync.dma_start(out=wkv_sb[:], in_=wkv.rearrange("(j p) o -> p j o", p=128))
            ones_sb = singles.tile([128, NK, H], BF16)
            nc.sync.dma_start(out=ones_sb[:], in_=ones.rearrange("(j p) o -> p j o", p=128))
            id_sb = singles.tile([128, 128], BF16)
            nc.sync.dma_start(out=id_sb[:], in_=ident)
            id12_sb = singles.tile([12, 12], BF16)
            nc.sync.dma_start(out=id12_sb[:], in_=id12)
            repl_sb = singles.tile([12, C], BF16)
            nc.sync.dma_start(out=repl_sb[:], in_=repl)
            wq_sb = singles.tile([128, NK, C], BF16)
            wp_sb = singles.tile([128, NK, C], BF16)
            bp_sb = singles.tile([128, NK], F32)
            xt_sb = singles.tile([128, NK, N], BF16)

            xr = xw.rearrange("(j p) n -> p j n", p=128)
            vpr = vp.rearrange("v (j p) n -> p j v n", p=128)

            def emit_late_consts():
                nc.sync.dma_start(out=xt_sb[:, :, HALF:N], in_=xr[:, :, HALF:N])
                nc.sync.dma_start(out=wp_sb[:], in_=wp.rearrange("(j p) o -> p j o", p=128))
                nc.sync.dma_start(out=bp_sb[:], in_=bp.rearrange("(j p) 1 -> p j", p=128))

            def emit_vp_loads(h2, g):
                # one tile per 4-variant group; short lifetime (KV proj only)
                T0 = h2 * HALF
                vpt = vtp_pool.tile([128, NK, 4, HALF], BF16, tag="vpg", name="vpg")
                for i in range(4):
                    nc.sync.dma_start(
                        out=vpt[:, :, i, :], in_=vpr[:, :, 4 * g + i, T0 : T0 + HALF]
                    )
                return vpt

            def emit_q_chunks(qt, h2, ms):
                T0 = h2 * HALF
                for m in ms:
                    psq = psum_mm.tile([128, HALF], F32, tag="mm", name="psq")
                    for k in range(NK):
                        nc.tensor.matmul(
                            psq[:],
                            lhsT=wq_sb[:, k, m * 128 : (m + 1) * 128],
                            rhs=xt_sb[:, k, T0 : T0 + HALF],
                            start=(k == 0),
                            stop=(k == NK - 1),
                        )
                    nc.scalar.copy(qt[:, m, :], psq[:])

            def emit_q(h2):
                qt = acts.tile([128, NK, HALF], BF16, tag="qt", name="qt")
                emit_q_chunks(qt, h2, range(NK))
                return qt

            def emit_kv_group(kvt, vpt, kv0, vp0, gn=2):
                # fused K+V: psum rows 0:64 = v, 64:128 = k (wkv pre-concat on
                # host); two parallel chains keep the PE streaming
                psks = [
                    psum_mm.tile([128, HALF], F32, tag="mm", name=f"pskv{i}")
                    for i in range(gn)
                ]
                for k in range(NK):
                    for i in range(gn):
                        nc.tensor.matmul(
                            psks[i][:],
                            lhsT=wkv_sb[:, k, :],
                            rhs=vpt[:, k, vp0 + i, :],
                            start=(k == 0),
                            stop=(k == NK - 1),
                        )
                for i in range(gn):
                    nc.scalar.copy(kvt[:, kv0 + i, :], psks[i][:])

            def emit_kt_dup(kvt, kt, vs0, vn):
                # duplicate the k rows (64:128 of kv) into both halves of kt
                nc.scalar.dma_start(
                    out=kt[0:64, vs0 : vs0 + vn, :], in_=kvt[64:128, vs0 : vs0 + vn, :]
                )
                nc.scalar.dma_start(
                    out=kt[64:128, vs0 : vs0 + vn, :], in_=kvt[64:128, vs0 : vs0 + vn, :]
                )

            def emit_tile(tt, qt, kt, kvt, ot):
                t0 = tt * 128
                # v natural: transpose v rows (kv[0:64]) per variant
                psvn = psum_tr.tile([128, C], BF16, tag="tr", name="psvn")
                for v in range(V):
                    nc.tensor.transpose(
                        psvn[:, v * HD : (v + 1) * HD],
                        kvt[0:64, v, t0 : t0 + 128],
                        id_sb[0:64, 0:64],
                    )
                vnat = sm_pool.tile([128, V, HD], BF16, name="vnat")
                nc.scalar.copy(vnat[:], psvn[:, 0 : V * HD])

                # QK -> s^T   (one 8-variant broadcast mul per k-chunk)
                psst = psum_s.tile([44, 512], F32, tag="ss", name="pss")
                pss1 = psst[0:12, :]
                pss2 = psst[32:44, :]
                for j in range(NK):
                    qa = qt[:, j, t0 : t0 + 128]
                    q_b = _bc(qa, [qa.ap[0], [0, V], qa.ap[-1]])
                    tmpa = tmp_pool.tile([128, V, 128], BF16, tag="tmpa", name="tmpa")
                    nc.vector.tensor_mul(tmpa[:], q_b, kt[:, :, t0 : t0 + 128])
                    nc.tensor.matmul(
                        pss1,
                        lhsT=ones_sb[:, j, :],
                        rhs=tmpa[:, 0:4, :],
                        start=(j == 0),
                        stop=(j == NK - 1),
                        tile_position=(0, 0),
                    )
                    nc.tensor.matmul(
                        pss2,
                        lhsT=ones_sb[:, j, :],
                        rhs=tmpa[:, 4:8, :],
                        start=(j == 0),
                        stop=(j == NK - 1),
                        tile_position=(0, 32),
                    )

                # softmax, token-major: exp -> e [12, V*128]; transpose each
                # variant's [12, 128] slab to [128, 12] -> eta [128 tok, (v,h)]
                e = sm_pool.tile([12, V * 128], BF16, name="e")
                nc.scalar.activation(e[:, 0:512], pss1, EXP, scale=SCALE)
                nc.scalar.activation(e[:, 512:1024], pss2, EXP, scale=SCALE)
                eta = psum_tr.tile([128, C], BF16, tag="tr", name="eta")
                for v in range(V):
                    nc.tensor.transpose(
                        eta[:, v * H : (v + 1) * H],
                        e[:, v * 128 : (v + 1) * 128],
                        id12_sb[:],
                    )
                z = sm_pool.tile([128, H], F32, name="z")
                ea = eta[:, 0 : V * H]
                e_sw = _bc(ea, [ea.ap[0], [1, H], [H, V]])
                nc.vector.tensor_reduce(
                    z[:], e_sw, axis=mybir.AxisListType.X, op=mybir.AluOpType.add
                )
                rz = sm_pool.tile([128, H], F32, name="rz")
                nc.vector.reciprocal(rz[:], z[:])

                # AV: head-replicate e per variant on the PE (straight into
                # PSUM, read by the DVE muls without any SBUF staging)
                ovA = av_pool.tile([128, 4, C], BF16, tag="ovA", name="ovA")
                ovB = av_pool.tile([128, 4, C], BF16, tag="ovB", name="ovB")
                ovs = (ovA, ovB)
                for v in range(V):
                    psxp = psum_tr.tile([128, C], BF16, tag="tr", name="psxp")
                    nc.tensor.transpose(
                        psxp[:, 0:384], e[:, v * 128 : (v + 1) * 128], repl_sb[:, 0:384]
                    )
                    nc.tensor.transpose(
                        psxp[:, 384:768],
                        e[:, v * 128 : (v + 1) * 128],
                        repl_sb[:, 384:768],
                    )
                    va = vnat[:, v, :]
                    vn_b = _bc(va, [va.ap[0], [0, H], va.ap[-1]])
                    nc.vector.tensor_mul(ovs[v // 4][:, v % 4, :], vn_b, psxp[:])
                # add tree: one paired level-1 add per ov tile, then combine
                for ov in ovs:
                    a0 = ov[:, 0:2:1, :]
                    in0 = _bc(a0, [a0.ap[0], [2 * C, 2], [1, C]])
                    a1 = ov[:, 1:3:1, :]
                    in1 = _bc(a1, [a1.ap[0], [2 * C, 2], [1, C]])
                    nc.vector.tensor_add(in0, in0, in1)
                    nc.vector.tensor_add(ov[:, 0, :], ov[:, 0, :], ov[:, 2, :])
                of = of_pool.tile([128, C], BF16, tag="of", name="of")
                nc.vector.tensor_add(of[:], ovA[:, 0, :], ovB[:, 0, :])
                # normalize once on the summed tile
                rza = rz[:]
                rz_b = _bc(rza, [rza.ap[0], [1, H], [0, HD]])
                nc.vector.tensor_mul(of[:], of[:], rz_b)

                # o^T
                psot = psum_tr.tile([128, C], BF16, tag="tr", name="psot")
                for j in range(NK):
                    nc.tensor.transpose(
                        psot[:, j * 128 : (j + 1) * 128],
                        of[:, j * 128 : (j + 1) * 128],
                        id_sb[:],
                    )
                oa = psot[:]
                nc.scalar.copy(
                    ot[:, :, t0 : t0 + 128], _bc(oa, [oa.ap[0], [128, NK], [1, 128]])
                )

            def emit_projout(h2, ot, ms=None):
                T0 = h2 * HALF
                for m in (range(NK) if ms is None else ms):
                    pso2 = psum_mm.tile([128, HALF], F32, tag="mm", name="pso2")
                    for k in range(NK):
                        nc.tensor.matmul(
                            pso2[:],
                            lhsT=wp_sb[:, k, m * 128 : (m + 1) * 128],
                            rhs=ot[:, k, :],
                            start=(k == 0),
                            stop=(k == NK - 1),
                        )
                    o2 = out_pool.tile([128, HALF], BF16, name="o2")
                    nc.scalar.activation(
                        o2[:], pso2[:], IDENT, bias=bp_sb[:, m : m + 1], scale=1.0
                    )
                    nc.gpsimd.dma_start(
                        out=outt[m * 128 : (m + 1) * 128, T0 : T0 + HALF], in_=o2[:]
                    )

            def emit_kv(h2, vpt, kvt=None, kt=None, phase=0):
                # vpt holds 4 variants (local idx 0..3) for group phase//2
                if kvt is None:
                    kvt = kv_pool.tile([128, V, HALF], BF16, tag="kv", name="kv")
                    kt = kt_pool.tile([128, V, HALF], BF16, tag="kt", name="kt")
                g0 = 0 if phase == 0 else 4
                emit_kv_group(kvt, vpt, g0, 0)
                emit_kv_group(kvt, vpt, g0 + 2, 2)
                emit_kt_dup(kvt, kt, g0, 4)
                return kvt, kt

            # ---- software-pipelined schedule over the two halves
            vpt00 = emit_vp_loads(0, 0)
            nc.sync.dma_start(out=xt_sb[:, :, 0:HALF], in_=xr[:, :, 0:HALF])
            nc.sync.dma_start(out=wq_sb[:], in_=wq.rearrange("(j p) o -> p j o", p=128))
            vpt01 = emit_vp_loads(0, 1)
            emit_late_consts()
            qt0 = acts.tile([128, NK, HALF], BF16, tag="qt", name="qt0")
            kv0, kt0 = emit_kv(0, vpt00, phase=0)
            emit_q_chunks(qt0, 0, [0])
            emit_kv(0, vpt01, kv0, kt0, phase=2)
            emit_q_chunks(qt0, 0, [1, 2, 3, 4, 5])
            vpt10 = emit_vp_loads(1, 0)
            vpt11 = emit_vp_loads(1, 1)
            ot0 = acts1.tile([128, NK, HALF], BF16, tag="ot", name="ot0")
            qt1 = kv1 = kt1 = None
            for tt in range(4):
                emit_tile(tt, qt0, kt0, kv0, ot0)
                if tt == 0:
                    qt1 = emit_q(1)
                elif tt == 1:
                    kv1, kt1 = emit_kv(1, vpt10, phase=0)
                elif tt == 2:
                    emit_kv(1, vpt11, kv1, kt1, phase=2)
            ot1 = acts1.tile([128, NK, HALF], BF16, tag="ot", name="ot1")
            emit_tile(0, qt1, kt1, kv1, ot1)
            emit_projout(0, ot0, [0, 1])
            emit_tile(1, qt1, kt1, kv1, ot1)
            emit_projout(0, ot0, [2, 3])
            emit_tile(2, qt1, kt1, kv1, ot1)
            emit_projout(0, ot0, [4, 5])
            emit_tile(3, qt1, kt1, kv1, ot1)
            emit_projout(1, ot1)

    _split_multi_waits(nc)
    return nc


_NC = None


def _get_nc():
    global _NC
    if _NC is None:
        _NC = build_kernel()
    return _NC


def _host_inputs(x, variants_patches, Wq, Wkv, Wproj, bproj):
    wq_t = np.ascontiguousarray(np.asarray(Wq, dtype=np.float32).T).astype(BF)
    wkv_np = np.asarray(Wkv, dtype=np.float32)
    # fused KV: psum rows 0:64 = v outputs, 64:128 = k outputs
    wkvcat = np.ascontiguousarray(
        np.concatenate([wkv_np[HD:], wkv_np[:HD]], axis=0).T
    ).astype(BF)
    wp_t = np.ascontiguousarray(np.asarray(Wproj, dtype=np.float32).T).astype(BF)
    bp = np.asarray(bproj, dtype=np.float32).reshape(C, 1)
    ones = np.zeros((C, H), dtype=np.float32)
    for c in range(C):
        ones[c, c // HD] = 1.0
    ones = ones.astype(BF)
    ident = np.eye(128, dtype=np.float32).astype(BF)
    id12_np = np.eye(12, dtype=np.float32).astype(BF)
    repl_np = np.zeros((H, C), dtype=np.float32)
    for c in range(C):
        repl_np[c // HD, c] = 1.0
    repl_np = repl_np.astype(BF)

    x = np.asarray(x, dtype=np.float32)
    vpn = np.asarray(variants_patches, dtype=np.float32)
    # pre-transpose activations on host: x -> [C, N], vp -> [V, C, N]
    xt = np.ascontiguousarray(x.transpose(0, 2, 1)).astype(BF)  # (B, C, N)
    vpt = np.ascontiguousarray(vpn.transpose(1, 0, 3, 2)).astype(BF)  # (B, V, C, N)
    in_maps = []
    for b in range(B):
        in_maps.append(
            {
                "xw": xt[b],
                "vp": vpt[b],
                "wq": wq_t,
                "wkv": wkvcat,
                "wp": wp_t,
                "bp": bp,
                "ones": ones,
                "ident": ident,
                "id12": id12_np,
                "repl": repl_np,
            }
        )
    return in_maps


def run(inputs, trace=False):
    nc = _get_nc()
    in_maps = _host_inputs(
        inputs["x"],
        inputs["variants_patches"],
        inputs["Wq"],
        inputs["Wkv"],
        inputs["Wproj"],
        inputs["bproj"],
    )
    res = run_bass_kernel_spmd(nc, in_maps, core_ids=list(range(8)), trace=trace)
    out = np.stack(
        [np.asarray(res.results[b]["outt"]).T for b in range(B)], axis=0
    ).astype(np.float32)
    return out, res


def kernel(**inputs) -> np.ndarray:
    out, _ = run(inputs, trace=False)
    return out


if __name__ == "__main__":
    rng = np.random.default_rng(0)
    ins = {
        "x": rng.standard_normal((B, N, C)).astype(np.float32),
        "variants_patches": rng.standard_normal((V, B, N, C)).astype(np.float32),
        "Wq": (rng.standard_normal((C, C)) * 0.02).astype(np.float32),
        "Wkv": (rng.standard_normal((2 * HD, C)) * 0.02).astype(np.float32),
        "Wproj": (rng.standard_normal((C, C)) * 0.02).astype(np.float32),
        "bproj": np.zeros((C,), dtype=np.float32),
        "num_layer": 0,
    }
    out = kernel(**ins)
    print("kernel ran, out shape", out.shape)


# revision 38
# speedup vs baseline: 1.2242x; 1.1164x over previous
"""Trainium2 Bass kernel for nn_AttentionModified (MQA-over-variants attention).

Strategy: data-parallel over B across 8 NeuronCores (no collectives — each
batch's output depends only on that batch's inputs).

Per-core pipeline (bf16 compute, f32 PSUM accumulation):
  - activations pre-transposed on host (x^T, vp^T) -> natural contiguous DMA
    loads (the on-device transpose-DMA wall of the earlier design is gone)
  - fused KV projection: one matmul chain per variant produces [v(64); k(64)]
    on 128 psum partitions (the M dim is free on the PE); k is then duplicated
    to the [k; k] layout QK needs via SBUF->SBUF DMAs issued from the scalar
    queue; v rows are consumed in place as a view
  - QK logits: one broadcast-AP vector multiply per k-chunk (q repeated over
    all 8 variants), then block-ones matmuls reduce 64-wide head groups -> s^T
  - softmax token-major: exp on the [12, 512] logit tiles, then cheap 12-col
    PE transposes -> [128 tok, (v,h)] psum; Z-reduce, reciprocal and the 1/Z
    scaling all happen on [128, 96] tiles (two orders of magnitude less work
    than head-major); per-variant head-replication to [128, 768] is split
    between the scalar and gpsimd engines
  - AV: paired broadcast-AP vector multiplies + a short add tree
  - output projection; bias fused into PSUM eviction; output written
    transposed, host transposes back
Emission order software-pipelines the two 512-token halves so PE projection
work for half h+1 fills the gaps in the DVE-bound attention phase of half h.
"""
import sys

sys.path.insert(0, "/opt/trn_rl_repo")

import numpy as np
import ml_dtypes

import concourse.bass as bass
import concourse.mybir as mybir
import concourse.tile as tile
from concourse.bass_utils import run_bass_kernel_spmd

BF16 = mybir.dt.bfloat16
F32 = mybir.dt.float32
BF = ml_dtypes.bfloat16

V, B, N, C, H = 8, 8, 1024, 768, 12
HD = C // H  # 64
NK = C // 128  # 6 contraction chunks
HALF = 512
SCALE = HD ** -0.5


def _split_multi_waits(nc):
    """This container's walrus accepts only one sync-wait per instruction;
    hoist extra waits onto same-engine NoOps inserted just before."""
    for f in nc.m.functions:
        for bb in f.blocks:
            new = []
            for inst in bb.instructions:
                si = inst.sync_info
                waits = list(si.on_wait) if (si and si.on_wait) else []
                if len(waits) > 1:
                    for i, w in enumerate(waits[:-1]):
                        nop = mybir.InstNoOp(name=f"{inst.name}-wsplit{i}")
                        nop.engine = inst.engine
                        nop.sync_info = mybir.SyncInfo(on_wait=[w], on_update=[])
                        new.append(nop)
                    si.on_wait = [waits[-1]]
                new.append(inst)
            bb.instructions[:] = new
    return nc


def _bc(a, dims):
    """Rebuild AP `a` with an explicit dim list (partition dim first)."""
    return bass.AP(tensor=a.tensor, offset=a.offset, ap=dims)


def build_kernel():
    nc = bass.Bass("TRN2", target_bir_lowering=False, debug=False, num_devices=8)

    xw = nc.dram_tensor("xw", [C, N], BF16, kind="ExternalInput").ap()
    vp = nc.dram_tensor("vp", [V, C, N], BF16, kind="ExternalInput").ap()
    wq = nc.dram_tensor("wq", [C, C], BF16, kind="ExternalInput").ap()
    wkv = nc.dram_tensor("wkv", [C, 128], BF16, kind="ExternalInput").ap()
    wp = nc.dram_tensor("wp", [C, C], BF16, kind="ExternalInput").ap()
    bp = nc.dram_tensor("bp", [C, 1], F32, kind="ExternalInput").ap()
    ones = nc.dram_tensor("ones", [C, H], BF16, kind="ExternalInput").ap()
    ident = nc.dram_tensor("ident", [128, 128], BF16, kind="ExternalInput").ap()
    id12 = nc.dram_tensor("id12", [12, 12], BF16, kind="ExternalInput").ap()
    repl = nc.dram_tensor("repl", [12, C], BF16, kind="ExternalInput").ap()
    outt = nc.dram_tensor("outt", [C, N], F32, kind="ExternalOutput").ap()

    EXP = mybir.ActivationFunctionType.Exp
    IDENT = mybir.ActivationFunctionType.Identity

    with tile.TileContext(nc) as tc:
        with (
            tc.tile_pool(name="singles", bufs=1) as singles,
            tc.tile_pool(name="vtp", bufs=3) as vtp_pool,
            tc.tile_pool(name="kvp", bufs=2) as kv_pool,
            tc.tile_pool(name="ktp", bufs=2) as kt_pool,
            tc.tile_pool(name="acts", bufs=2) as acts,
            tc.tile_pool(name="acts1", bufs=2) as acts1,
            tc.tile_pool(name="tmp", bufs=2) as tmp_pool,
            tc.tile_pool(name="sm", bufs=2) as sm_pool,
            tc.tile_pool(name="av", bufs=1) as av_pool,
            tc.tile_pool(name="of", bufs=2) as of_pool,
            tc.tile_pool(name="outp", bufs=2) as out_pool,
            tc.tile_pool(name="psmm", bufs=3, space="PSUM") as psum_mm,
            tc.tile_pool(name="psss", bufs=2, space="PSUM") as psum_s,
            tc.tile_pool(name="pstr", bufs=3, space="PSUM") as psum_tr,
        ):
            # ---- constants (emission order = sync-ring order: small KV weight
            # first, then vp chunks so the fused KV proj can start early)
            wkv_sb = singles.tile([128, NK, 128], BF16)
            nc.sync.dma_start(out=wkv_sb[:], in_=wkv.rearrange("(j p) o -> p j o", p=128))
            ones_sb = singles.tile([128, NK, H], BF16)
            nc.sync.dma_start(out=ones_sb[:], in_=ones.rearrange("(j p) o -> p j o", p=128))
            id_sb = singles.tile([128, 128], BF16)
            nc.sync.dma_start(out=id_sb[:], in_=ident)
            id12_sb = singles.tile([12, 12], BF16)
            nc.sync.dma_start(out=id12_sb[:], in_=id12)
            repl_sb = singles.tile([12, C], BF16)
            nc.sync.dma_start(out=repl_sb[:], in_=repl)
            wq_sb = singles.tile([128, NK, C], BF16)
            wp_sb = singles.tile([128, NK, C], BF16)
            bp_sb = singles.tile([128, NK], F32)
            xt_sb = singles.tile([128, NK, N], BF16)

            xr = xw.rearrange("(j p) n -> p j n", p=128)
            vpr = vp.rearrange("v (j p) n -> p j v n", p=128)

            def emit_late_consts():
                nc.sync.dma_start(out=xt_sb[:, :, HALF:N], in_=xr[:, :, HALF:N])
                nc.sync.dma_start(out=wp_sb[:], in_=wp.rearrange("(j p) o -> p j o", p=128))
                nc.sync.dma_start(out=bp_sb[:], in_=bp.rearrange("(j p) 1 -> p j", p=128))

            def emit_vp_loads(h2, g):
                # one tile per 4-variant group; short lifetime (KV proj only)
                T0 = h2 * HALF
                vpt = vtp_pool.tile([128, NK, 4, HALF], BF16, tag="vpg", name="vpg")
                for i in range(4):
                    nc.sync.dma_start(
                        out=vpt[:, :, i, :], in_=vpr[:, :, 4 * g + i, T0 : T0 + HALF]
                    )
                return vpt

            def emit_q_chunks(qt, h2, ms):
                T0 = h2 * HALF
                for m in ms:
                    psq = psum_mm.tile([128, HALF], F32, tag="mm", name="psq")
                    for k in range(NK):
                        nc.tensor.matmul(
                            psq[:],
                            lhsT=wq_sb[:, k, m * 128 : (m + 1) * 128],
                            rhs=xt_sb[:, k, T0 : T0 + HALF],
                            start=(k == 0),
                            stop=(k == NK - 1),
                        )
                    nc.scalar.copy(qt[:, m, :], psq[:])

            def emit_q(h2):
                qt = acts.tile([128, NK, HALF], BF16, tag="qt", name="qt")
                emit_q_chunks(qt, h2, range(NK))
                return qt

            def emit_kv_chain(kvt, vpt, kv0, vp0):
                # fused K+V: psum rows 0:64 = v, 64:128 = k (wkv pre-concat
                # on host); one accumulation chain per variant
                psk = psum_mm.tile([128, HALF], F32, tag="mm", name="pskv")
                for k in range(NK):
                    nc.tensor.matmul(
                        psk[:],
                        lhsT=wkv_sb[:, k, :],
                        rhs=vpt[:, k, vp0, :],
                        start=(k == 0),
                        stop=(k == NK - 1),
                    )
                nc.scalar.copy(kvt[:, kv0, :], psk[:])

            def emit_kt_dup(kvt, kt, vs0, vn):
                # duplicate the k rows (64:128 of kv) into both halves of kt
                nc.scalar.dma_start(
                    out=kt[0:64, vs0 : vs0 + vn, :], in_=kvt[64:128, vs0 : vs0 + vn, :]
                )
                nc.scalar.dma_start(
                    out=kt[64:128, vs0 : vs0 + vn, :], in_=kvt[64:128, vs0 : vs0 + vn, :]
                )

            def emit_tile(tt, qt, kt, kvt, ot, bgq=None):
                # bgq: deque of background PE-chain closures (projections for
                # the other half) emitted at the tile's natural PE gap points
                # so the tensor engine never idles / drops p-state
                def bg():
                    if bgq:
                        bgq.popleft()()

                t0 = tt * 128
                # v natural: transpose v rows (kv[0:64]) per variant
                psvn = psum_tr.tile([128, C], BF16, tag="tr", name="psvn")
                for v in range(V):
                    nc.tensor.transpose(
                        psvn[:, v * HD : (v + 1) * HD],
                        kvt[0:64, v, t0 : t0 + 128],
                        id_sb[0:64, 0:64],
                    )
                vnat = sm_pool.tile([128, V, HD], BF16, name="vnat")
                nc.scalar.copy(vnat[:], psvn[:, 0 : V * HD])
                bg()

                # QK -> s^T   (one 8-variant broadcast mul per k-chunk)
                psst = psum_s.tile([44, 512], F32, tag="ss", name="pss")
                pss1 = psst[0:12, :]
                pss2 = psst[32:44, :]
                for j in range(NK):
                    if j == 3:
                        bg()
                    qa = qt[:, j, t0 : t0 + 128]
                    q_b = _bc(qa, [qa.ap[0], [0, V], qa.ap[-1]])
                    tmpa = tmp_pool.tile([128, V, 128], BF16, tag="tmpa", name="tmpa")
                    nc.vector.tensor_mul(tmpa[:], q_b, kt[:, :, t0 : t0 + 128])
                    nc.tensor.matmul(
                        pss1,
                        lhsT=ones_sb[:, j, :],
                        rhs=tmpa[:, 0:4, :],
                        start=(j == 0),
                        stop=(j == NK - 1),
                        tile_position=(0, 0),
                    )
                    nc.tensor.matmul(
                        pss2,
                        lhsT=ones_sb[:, j, :],
                        rhs=tmpa[:, 4:8, :],
                        start=(j == 0),
                        stop=(j == NK - 1),
                        tile_position=(0, 32),
                    )

                # softmax, token-major: exp -> e [12, V*128]; transpose each
                # variant's [12, 128] slab to [128, 12] -> eta [128 tok, (v,h)]
                bg()
                e = sm_pool.tile([12, V * 128], BF16, name="e")
                nc.scalar.activation(e[:, 0:512], pss1, EXP, scale=SCALE)
                nc.scalar.activation(e[:, 512:1024], pss2, EXP, scale=SCALE)
                eta = psum_tr.tile([128, C], BF16, tag="tr", name="eta")
                for v in range(V):
                    nc.tensor.transpose(
                        eta[:, v * H : (v + 1) * H],
                        e[:, v * 128 : (v + 1) * 128],
                        id12_sb[:],
                    )
                bg()
                z = sm_pool.tile([128, H], F32, name="z")
                ea = eta[:, 0 : V * H]
                e_sw = _bc(ea, [ea.ap[0], [1, H], [H, V]])
                nc.vector.tensor_reduce(
                    z[:], e_sw, axis=mybir.AxisListType.X, op=mybir.AluOpType.add
                )
                rz = sm_pool.tile([128, H], F32, name="rz")
                nc.vector.reciprocal(rz[:], z[:])

                # AV: head-replicate e per variant on the PE (straight into
                # PSUM, read by the DVE muls without any SBUF staging)
                ovA = av_pool.tile([128, 4, C], BF16, tag="ovA", name="ovA")
                ovB = av_pool.tile([128, 4, C], BF16, tag="ovB", name="ovB")
                ovs = (ovA, ovB)
                for v in range(V):
                    if v == 4:
                        bg()
                    psxp = psum_tr.tile([128, C], BF16, tag="tr", name="psxp")
                    nc.tensor.transpose(
                        psxp[:, 0:384], e[:, v * 128 : (v + 1) * 128], repl_sb[:, 0:384]
                    )
                    nc.tensor.transpose(
                        psxp[:, 384:768],
                        e[:, v * 128 : (v + 1) * 128],
                        repl_sb[:, 384:768],
                    )
                    va = vnat[:, v, :]
                    vn_b = _bc(va, [va.ap[0], [0, H], va.ap[-1]])
                    nc.vector.tensor_mul(ovs[v // 4][:, v % 4, :], vn_b, psxp[:])
                # add tree: one paired level-1 add per ov tile, then combine
                for ov in ovs:
                    a0 = ov[:, 0:2:1, :]
                    in0 = _bc(a0, [a0.ap[0], [2 * C, 2], [1, C]])
                    a1 = ov[:, 1:3:1, :]
                    in1 = _bc(a1, [a1.ap[0], [2 * C, 2], [1, C]])
                    nc.vector.tensor_add(in0, in0, in1)
                    nc.vector.tensor_add(ov[:, 0, :], ov[:, 0, :], ov[:, 2, :])
                of = of_pool.tile([128, C], BF16, tag="of", name="of")
                nc.vector.tensor_add(of[:], ovA[:, 0, :], ovB[:, 0, :])
                # normalize once on the summed tile
                rza = rz[:]
                rz_b = _bc(rza, [rza.ap[0], [1, H], [0, HD]])
                nc.vector.tensor_mul(of[:], of[:], rz_b)

                # o^T
                psot = psum_tr.tile([128, C], BF16, tag="tr", name="psot")
                for j in range(NK):
                    nc.tensor.transpose(
                        psot[:, j * 128 : (j + 1) * 128],
                        of[:, j * 128 : (j + 1) * 128],
                        id_sb[:],
                    )
                oa = psot[:]
                nc.scalar.copy(
                    ot[:, :, t0 : t0 + 128], _bc(oa, [oa.ap[0], [128, NK], [1, 128]])
                )

            def emit_projout(h2, ot, ms=None):
                T0 = h2 * HALF
                for m in (range(NK) if ms is None else ms):
                    pso2 = psum_mm.tile([128, HALF], F32, tag="mm", name="pso2")
                    for k in range(NK):
                        nc.tensor.matmul(
                            pso2[:],
                            lhsT=wp_sb[:, k, m * 128 : (m + 1) * 128],
                            rhs=ot[:, k, :],
                            start=(k == 0),
                            stop=(k == NK - 1),
                        )
                    o2 = out_pool.tile([128, HALF], BF16, name="o2")
                    nc.scalar.activation(
                        o2[:], pso2[:], IDENT, bias=bp_sb[:, m : m + 1], scale=1.0
                    )
                    nc.gpsimd.dma_start(
                        out=outt[m * 128 : (m + 1) * 128, T0 : T0 + HALF], in_=o2[:]
                    )

            def kv_tiles():
                kvt = kv_pool.tile([128, V, HALF], BF16, tag="kv", name="kv")
                kt = kt_pool.tile([128, V, HALF], BF16, tag="kt", name="kt")
                return kvt, kt

            def kv_closures(kvt, kt, vptA, vptB):
                cl = []
                for g, vpt in ((0, vptA), (4, vptB)):
                    for i in range(4):
                        cl.append(
                            lambda kvt=kvt, vpt=vpt, g=g, i=i: emit_kv_chain(
                                kvt, vpt, g + i, i
                            )
                        )
                    cl.append(lambda kvt=kvt, kt=kt, g=g: emit_kt_dup(kvt, kt, g, 4))
                return cl

            # ---- software-pipelined schedule over the two halves
            from collections import deque

            vpt00 = emit_vp_loads(0, 0)
            nc.sync.dma_start(out=xt_sb[:, :, 0:HALF], in_=xr[:, :, 0:HALF])
            nc.sync.dma_start(out=wq_sb[:], in_=wq.rearrange("(j p) o -> p j o", p=128))
            vpt01 = emit_vp_loads(0, 1)
            emit_late_consts()
            qt0 = acts.tile([128, NK, HALF], BF16, tag="qt", name="qt0")
            kv0, kt0 = kv_tiles()
            for cl in kv_closures(kv0, kt0, vpt00, vpt01)[:5]:
                cl()
            emit_q_chunks(qt0, 0, [0])
            for cl in kv_closures(kv0, kt0, vpt00, vpt01)[5:]:
                cl()
            emit_q_chunks(qt0, 0, [1, 2, 3, 4, 5])
            vpt10 = emit_vp_loads(1, 0)
            vpt11 = emit_vp_loads(1, 1)
            ot0 = acts1.tile([128, NK, HALF], BF16, tag="ot", name="ot0")
            # background for half-0's attention tiles: Q1 + KV1 chains
            qt1 = acts.tile([128, NK, HALF], BF16, tag="qt", name="qt1")
            kv1, kt1 = kv_tiles()
            bgq = deque()
            bgq.extend(kv_closures(kv1, kt1, vpt10, vpt11))
            for m in range(NK):
                bgq.append(lambda m=m: emit_q_chunks(qt1, 1, [m]))
            for tt in range(4):
                emit_tile(tt, qt0, kt0, kv0, ot0, bgq)
            while bgq:
                bgq.popleft()()
            # background for half-1's attention tiles: proj of half 0
            ot1 = acts1.tile([128, NK, HALF], BF16, tag="ot", name="ot1")
            bgq = deque()
            for m in range(NK):
                bgq.append(lambda m=m: emit_projout(0, ot0, [m]))
            for tt in range(4):
                emit_tile(tt, qt1, kt1, kv1, ot1, bgq)
            while bgq:
                bgq.popleft()()
            emit_projout(1, ot1)

    _split_multi_waits(nc)
    return nc


_NC = None


def _get_nc():
    global _NC
    if _NC is None:
        _NC = build_kernel()
    return _NC


def _host_inputs(x, variants_patches, Wq, Wkv, Wproj, bproj):
    wq_t = np.ascontiguousarray(np.asarray(Wq, dtype=np.float32).T).astype(BF)
    wkv_np = np.asarray(Wkv, dtype=np.float32)
    # fused KV: psum rows 0:64 = v outputs, 64:128 = k outputs
    wkvcat = np.ascontiguousarray(
        np.concatenate([wkv_np[HD:], wkv_np[:HD]], axis=0).T
    ).astype(BF)
    wp_t = np.ascontiguousarray(np.asarray(Wproj, dtype=np.float32).T).astype(BF)
    bp = np.asarray(bproj, dtype=np.float32).reshape(C, 1)
    ones = np.zeros((C, H), dtype=np.float32)
    for c in range(C):
        ones[c, c // HD] = 1.0
    ones = ones.astype(BF)
    ident = np.eye(128, dtype=np.float32).astype(BF)
    id12_np = np.eye(12, dtype=np.float32).astype(BF)
    repl_np = np.zeros((H, C), dtype=np.float32)
    for c in range(C):
        repl_np[c // HD, c] = 1.0
    repl_np = repl_np.astype(BF)

    x = np.asarray(x, dtype=np.float32)
    vpn = np.asarray(variants_patches, dtype=np.float32)
    # pre-transpose activations on host: x -> [C, N], vp -> [V, C, N]
    xt = np.ascontiguousarray(x.transpose(0, 2, 1)).astype(BF)  # (B, C, N)
    vpt = np.ascontiguousarray(vpn.transpose(1, 0, 3, 2)).astype(BF)  # (B, V, C, N)
    in_maps = []
    for b in range(B):
        in_maps.append(
            {
                "xw": xt[b],
                "vp": vpt[b],
                "wq": wq_t,
                "wkv": wkvcat,
                "wp": wp_t,
                "bp": bp,
                "ones": ones,
                "ident": ident,
                "id12": id12_np,
                "repl": repl_np,
            }
        )
    return in_maps


def run(inputs, trace=False):
    nc = _get_nc()
    in_maps = _host_inputs(
        inputs["x"],
        inputs["variants_patches"],
        inputs["Wq"],
        inputs["Wkv"],
        inputs["Wproj"],
        inputs["bproj"],
    )
    res = run_bass_kernel_spmd(nc, in_maps, core_ids=list(range(8)), trace=trace)
    out = np.stack(
        [np.asarray(res.results[b]["outt"]).T for b in range(B)], axis=0
    ).astype(np.float32)
    return out, res


def kernel(**inputs) -> np.ndarray:
    out, _ = run(inputs, trace=False)
    return out


if __name__ == "__main__":
    rng = np.random.default_rng(0)
    ins = {
        "x": rng.standard_normal((B, N, C)).astype(np.float32),
        "variants_patches": rng.standard_normal((V, B, N, C)).astype(np.float32),
        "Wq": (rng.standard_normal((C, C)) * 0.02).astype(np.float32),
        "Wkv": (rng.standard_normal((2 * HD, C)) * 0.02).astype(np.float32),
        "Wproj": (rng.standard_normal((C, C)) * 0.02).astype(np.float32),
        "bproj": np.zeros((C,), dtype=np.float32),
        "num_layer": 0,
    }
    out = kernel(**ins)
    print("kernel ran, out shape", out.shape)
